# revision 10
# baseline (speedup 1.0000x reference)
"""Trainium2 Bass kernel for nn_Block_41077067219413.

Reference computation (B=2048, D=dim_in=4096, J=dim_out=4096):
    xf = x.astype(f32)                 # (B, D) in {0,1}
    mf = masks.astype(f32)             # (D, J) in {0,1}
    sums = xf @ mf + (1-xf) @ (1-mf)   # XNOR popcount over D
    out  = sums > thresholds[None, :]  # (B, J) bool

Identity: with x' = 2x-1 in {-1,+1} and m in {0,1},
    A[j,b] = sum_k m[k,j] * x'[k,b]
    sums   = A + D - rowsum_x[b]
    out    = A > th[j] + rowsum_x[b] - D

Sharding: tensor-parallel over dim_out (each core owns J/8 = 512 output
columns).  masks (fp8 {0,1}) are the stationary operand; x is transposed
and converted to fp8 {-1,+1} on the host and streamed as the moving
operand -- no on-device transposes and only ~11MB of DMA per core (vs
21.8MB for batch-parallel with replicated masks).  DoubleRowSwInterleave
with host-pre-interleaved weights.  Epilogue: DVE is_gt of PSUM vs
R2d[j,b] = th[j] + rowsum_x[b] - D (built on the Scalar engine).  All
arithmetic is exact in fp32.
"""

import numpy as np

B, D, J = 2048, 4096, 4096
NCORES = 8
P = 128
GB, GJ = 1, 8             # core grid: batch x dim_out
assert GB * GJ == NCORES
BL = B // GB              # batch per core (moving free dim)
JL = J // GJ              # out columns per core
KP = D // 256             # 16 DoubleRow pair-blocks of 256 k
CHUNKS = [1, 1, 2, 2, 2, 2, 2, 2, 2]   # kp per DMA chunk
assert sum(CHUNKS) == KP
JBLK = JL // P            # j-blocks of 128 (psum partition dim)
NBC = BL // 512           # moving chunks of 512 (one psum bank each)
NACCW = 8 // NBC          # j-blocks in flight per wave
NWAVE = JBLK // NACCW

_cache = {}


def _build():
    import concourse.bacc as bacc
    import concourse.mybir as mybir
    import concourse.tile as tile

    dt = mybir.dt
    f8 = dt.float8e4
    f32 = dt.float32
    AF = mybir.ActivationFunctionType
    ALU = mybir.AluOpType
    DR = mybir.MatmulPerfMode.DoubleRowSwInterleave

    nc = bacc.Bacc("TRN2", target_bir_lowering=False, debug=False,
                   num_devices=NCORES)

    x_d = nc.dram_tensor("x8", [KP, P, 2, BL], dt.uint8,
                         kind="ExternalInput")
    # masks pre-interleaved on host for DoubleRowSwInterleave weight loads
    m_d = nc.dram_tensor("m8", [KP, P, JBLK, P, 2], dt.uint8,
                         kind="ExternalInput")
    rbc_d = nc.dram_tensor("rbc", [P, BL], f32, kind="ExternalInput")
    thc_d = nc.dram_tensor("thc", [P, JBLK], f32, kind="ExternalInput")
    o_d = nc.dram_tensor("out", [JL, BL], dt.uint8, kind="ExternalOutput")

    with tile.TileContext(nc) as tc:
        with (
            tc.tile_pool(name="const", bufs=1) as constp,
            tc.tile_pool(name="xp", bufs=1) as xp,
            tc.tile_pool(name="mp", bufs=1) as mp,
            tc.tile_pool(name="r2dp", bufs=1) as r2dp,
            tc.tile_pool(name="obp", bufs=1) as obp,
        ):
            # bulk input DMAs lead the gpsimd (SWDGE) queue, interleaved
            # by kp chunk so compute can start as soon as chunk 0 lands.
            # The first x chunk is further split by moving column block.
            m_t = {}   # kp -> (tile, k2)
            x_t = {}   # (kp, bc) -> (tile, k2, bc_in_tile)
            kp0 = 0
            for ci, kch in enumerate(CHUNKS):
                mt = mp.tile([P, kch, JBLK, P, 2], dt.uint8,
                             name=f"m{ci}", tag=f"m{ci}")
                nc.gpsimd.dma_start(mt[:], m_d[kp0:kp0 + kch].rearrange(
                    "kp ki jb m two -> ki kp jb m two"))
                for k2 in range(kch):
                    m_t[kp0 + k2] = (mt, k2)
                if ci == 0:
                    for bc in range(NBC):
                        xt = xp.tile([P, kch, 2, 512], dt.uint8,
                                     name=f"x{ci}_{bc}", tag=f"x{ci}_{bc}")
                        nc.gpsimd.dma_start(
                            xt[:],
                            x_d[kp0:kp0 + kch, :, :,
                                bc * 512:(bc + 1) * 512].rearrange(
                                "kp ki ko b -> ki kp ko b"))
                        for k2 in range(kch):
                            x_t[(kp0 + k2, bc)] = (xt, k2, 0)
                else:
                    xt = xp.tile([P, kch, 2, BL], dt.uint8,
                                 name=f"x{ci}", tag=f"x{ci}")
                    nc.gpsimd.dma_start(xt[:], x_d[kp0:kp0 + kch].rearrange(
                        "kp ki ko b -> ki kp ko b"))
                    for k2 in range(kch):
                        for bc in range(NBC):
                            x_t[(kp0 + k2, bc)] = (xt, k2, bc)
                kp0 += kch

            # small consts on the scalar (HWDGE) queue
            rbc = constp.tile([P, BL], f32)
            nc.scalar.dma_start(rbc[:], rbc_d[:])
            thc = constp.tile([P, JBLK], f32)
            nc.scalar.dma_start(thc[:], thc_d[:])

            # scalar-engine act table warmup
            neg1 = constp.tile([P, 1], f32)
            nc.vector.memset(neg1[:], -1.0)
            actwarm = constp.tile([P, 1], f32)
            nc.scalar.activation(actwarm[:], neg1[:], AF.Identity,
                                 bias=neg1[:], scale=1.0)

            # R2d[p, jb, b] = th[jb*128+p] + rowsum[b] - D  (scalar engine)
            r2d = r2dp.tile([P, JBLK, BL], f32)
            for jb in range(JBLK):
                nc.scalar.activation(r2d[:, jb, :], rbc[:], AF.Identity,
                                     bias=thc[:, jb:jb + 1], scale=1.0)

            obs = [obp.tile([P, BL], dt.uint8, name=f"ob{jb}", tag=f"ob{jb}")
                   for jb in range(JBLK)]

            # PE clock warmup: dummy matmuls on memset data while the
            # first input chunks are still in flight (HAM needs ~3us of
            # continuous PE-busy to reach the full 2.4GHz p-state)
            warm8 = constp.tile([P, 640], dt.uint8)
            nc.vector.memset(warm8[:], 0)
            with tc.tile_pool(name="pswarm", bufs=1, space="PSUM") as pswarm:
                wps = pswarm.tile([P, 512], f32)
                for _ in range(10):
                    nc.tensor.matmul(wps[:], warm8[:, 0:P].bitcast(f8),
                                     warm8[:, P:P + 512].bitcast(f8),
                                     start=True, stop=True)

            with tc.tile_pool(name="psacc", bufs=1, space="PSUM") as psacc:
                for w in range(NWAVE):
                    jbs = [w * NACCW + i for i in range(NACCW)]
                    accs = {}
                    for i, jb in enumerate(jbs):
                        for bc in range(NBC):
                            accs[(jb, bc)] = psacc.tile(
                                [P, 512], f32,
                                name=f"acc_w{w}_{jb}_{bc}",
                                tag=f"acc{i}_{bc}")
                    # kp-major accumulation; last kp + epilogue emitted
                    # acc-major so epilogues stagger with the other
                    # accumulators' remaining matmuls
                    for kp in range(KP - 1):
                        mt, mk2 = m_t[kp]
                        for jb in jbs:
                            for bc in range(NBC):
                                xt, xk2, xbc = x_t[(kp, bc)]
                                nc.tensor.matmul(
                                    accs[(jb, bc)][:],
                                    mt[:, mk2, jb, :, :].bitcast(f8),
                                    xt[:, xk2, :,
                                       xbc * 512:(xbc + 1) * 512]
                                    .bitcast(f8),
                                    start=(kp == 0), stop=False,
                                    perf_mode=DR)
                    kp = KP - 1
                    mt, mk2 = m_t[kp]
                    for jb in jbs:
                        for bc in range(NBC):
                            xt, xk2, xbc = x_t[(kp, bc)]
                            nc.tensor.matmul(
                                accs[(jb, bc)][:],
                                mt[:, mk2, jb, :, :].bitcast(f8),
                                xt[:, xk2, :, xbc * 512:(xbc + 1) * 512]
                                .bitcast(f8),
                                start=False, stop=True, perf_mode=DR)
                        for bc in range(NBC):
                            nc.vector.tensor_tensor(
                                obs[jb][:, bc * 512:(bc + 1) * 512],
                                accs[(jb, bc)][:],
                                r2d[:, jb, bc * 512:(bc + 1) * 512],
                                op=ALU.is_gt)
                        nc.gpsimd.dma_start(o_d[jb * P:(jb + 1) * P, :],
                                            obs[jb][:])

    nc.compile()
    return nc


def _get_nc():
    if "nc" not in _cache:
        _cache["nc"] = _build()
    return _cache["nc"]


def _prep(x, masks, thresholds):
    """Host-side shard/layout prep for the 8 cores."""
    xb = (np.asarray(x) != 0)
    # x' = 2x-1 as fp8e4m3 bytes: +1 -> 0x38, -1 -> 0xB8; transposed [D, B]
    xT8 = np.where(xb.T, np.uint8(0x38), np.uint8(0xB8))
    m_u8 = np.asarray(masks).view(np.uint8)
    rowsum = xb.sum(axis=1, dtype=np.int64).astype(np.float32)
    th = np.asarray(thresholds).astype(np.float32)

    in_maps = []
    for core in range(NCORES):
        bi, ji = core // GJ, core % GJ
        bsl = slice(bi * BL, (bi + 1) * BL)
        jsl = slice(ji * JL, (ji + 1) * JL)
        # k = (kp*2 + ko)*128 + ki  ->  [kp, ki, ko, b]
        x8 = np.ascontiguousarray(
            xT8[:, bsl].reshape(KP, 2, P, BL).transpose(0, 2, 1, 3))
        # SwInterleave weight layout: per partition ki the 256 stationary
        # bytes are [A_{m127}, B_{m127}, ..., A_{m0}, B_{m0}] (A/B = the
        # two k-halves of the DoubleRow pair, m reversed)
        m8 = (m_u8[:, jsl] * np.uint8(0x38)) \
            .reshape(KP, 2, P, JBLK, P)          # kp ko ki jb m
        m8 = np.ascontiguousarray(
            m8[:, :, :, :, ::-1].transpose(0, 2, 3, 4, 1))
        rbc = np.ascontiguousarray(
            np.broadcast_to(rowsum[bsl] - np.float32(D), (P, BL)))
        thc = np.ascontiguousarray(th[jsl].reshape(JBLK, P).T)
        in_maps.append({"x8": x8, "m8": m8, "rbc": rbc, "thc": thc})
    return in_maps


def run(x, masks, thresholds, trace=False):
    """Run the SPMD kernel on 8 cores. Returns (out_bool, BassKernelResults)."""
    from concourse.bass_utils import run_bass_kernel_spmd

    nc = _get_nc()
    in_maps = _prep(x, masks, thresholds)
    res = run_bass_kernel_spmd(nc, in_maps, core_ids=list(range(NCORES)),
                               trace=trace)
    out = np.empty((B, J), dtype=np.uint8)
    for core in range(NCORES):
        bi, ji = core // GJ, core % GJ
        out[bi * BL:(bi + 1) * BL, ji * JL:(ji + 1) * JL] = \
            res.results[core]["out"].T
    return out.view(np.bool_), res


def kernel(x, masks, thresholds):
    x = np.asarray(x)
    masks = np.asarray(masks)
    thresholds = np.asarray(thresholds)
    out, _ = run(x, masks, thresholds, trace=False)
    return out
